# revision 7
# baseline (speedup 1.0000x reference)
"""Multi-head attention with fraction-based RoPE ("stoich RoPE") on 8
Trainium2 NeuronCores.

Sharding: each core owns one (batch, query-half) pair — B=4 batches x 2
query halves = 8 shards.  Every core projects Q for its 1024 query rows
and K/V for the full 2048 keys of its batch (K/V projection is computed
on both cores sharing a batch; the 2x redundancy buys a kernel with no
collectives: the attention output rows owned by a core carry the full
head dimension, so the output projection and bias are entirely local).

Per-core device program (SPMD, identical on all 8 cores):
  phase A  per head-pair (8 x 128 dims): project Q^T/K^T/V^T from x^T
           streamed out of DRAM (weights stationary, x moving), add
           biases, apply RoPE to Q/K via precomputed cos/sin tiles and
           32-partition cross-quadrant swaps, PE-transpose V into
           natural layout with a ones column appended (row 64 of the
           P@V' output then carries the softmax denominator).
  phase B  attention per head: scores^T = K^T.T @ Q^T chunks -> exp on
           ACT (scale=1/8 folded in, no max subtraction: |scores/8| is
           O(1) for this operator's input distribution) -> P^T@V'
           accumulation -> reciprocal + K=1 broadcast matmul ->
           normalized attn^T written per pair region.
  phase C  output projection: attn^T chunks stationary, Wo^T moving,
           + bias, DMA out rows.

The host shards/formats inputs (transposes, bias/cos-sin tiles) and
concatenates the 8 output row-shards.
"""

import contextlib
import ctypes
import sys
import types

import numpy as np
import ml_dtypes

import concourse.bass as bass
import concourse.mybir as mybir
import concourse.tile as tile
from concourse.masks import make_identity
from concourse.vector_clock import ScopedClock

# ---------------- problem constants (hardcoded per contract) ----------------
B, T, D = 4, 2048, 1024
H, HD = 16, 64  # heads, head dim
HALF = HD // 2
N_CORES = 8
TQ = T // 2  # query rows per core
P = 128
NQ = 512  # moving-dim tile for matmuls
NPAIR = D // P  # 8 head pairs per core
SCALE = 1.0 / np.sqrt(HD)  # folded into exp()
ROPE_SCALE = 1000.0
ROPE_BASE = 10000.0

F32 = mybir.dt.float32
DT_MM = mybir.dt.bfloat16  # dtype of matmul operands (bfloat16 | float32)

_SO_PATH = "/opt/axon/libaxon_pjrt.so"


# ---------------- axon/NTFF environment shims ----------------
def _ntff_profile_hook():
    try:
        lib = ctypes.CDLL(_SO_PATH)
    except OSError:
        return None
    if not hasattr(lib, "axon_start_nrt_profile"):
        return None
    lib.axon_start_nrt_profile.argtypes = [
        ctypes.POINTER(ctypes.c_int64),
        ctypes.c_size_t,
    ]
    lib.axon_start_nrt_profile.restype = ctypes.c_int64
    lib.axon_stop_nrt_profile.argtypes = [ctypes.c_char_p]
    lib.axon_stop_nrt_profile.restype = ctypes.c_int64

    @contextlib.contextmanager
    def _hook(output_dir, device_ids):
        import jax

        jax.devices()
        if device_ids:
            ids = (ctypes.c_int64 * len(device_ids))(*device_ids)
            rc = lib.axon_start_nrt_profile(ids, len(device_ids))
        else:
            rc = lib.axon_start_nrt_profile(None, 0)
        if rc != 0:
            raise RuntimeError(f"axon_start_nrt_profile rc={rc}")
        try:
            yield
        finally:
            n = lib.axon_stop_nrt_profile(str(output_dir).encode())
            if n < 0:
                raise RuntimeError(f"axon_stop_nrt_profile rc={n}")

    return _hook


def install_shims():
    if "antenv.axon_hooks" not in sys.modules:
        mod = types.ModuleType("antenv.axon_hooks")
        hook = _ntff_profile_hook()
        mod.get_axon_ntff_profile_hook = lambda: hook
        mod.set_axon_ntff_profile_hook = lambda h: None
        sys.modules["antenv.axon_hooks"] = mod
    import concourse.bass_utils as bass_utils

    bass_utils.upload_artifacts = lambda tmpdir: str(tmpdir)


class TileContextSplitDrain(tile.TileContext):
    """This walrus build encodes at most 2 sync waits per CTRL
    instruction; Tile's kernel-tail drain wants one wait per logical
    processor.  Split the waits across single-wait NOPs instead."""

    MAX_WAITS = 1

    def _drain_and_barrier(self, tick_clock, wait_clock):
        nc = self.nc
        carrier = nc.sync.nop(nofuse=True)
        wait_clock.add_sem_waits(
            carrier.ins, ScopedClock({None: tick_clock.global_clock})
        )
        waits = list(carrier.ins.sync_info.on_wait or [])
        if len(waits) > self.MAX_WAITS:
            carrier.ins.sync_info.on_wait[:] = waits[: self.MAX_WAITS]
            for i in range(self.MAX_WAITS, len(waits), self.MAX_WAITS):
                extra = nc.sync.nop(nofuse=True)
                extra.ins.sync_info = mybir.SyncInfo(
                    on_wait=list(waits[i : i + self.MAX_WAITS]), on_update=[]
                )
        nc.sync.drain()
        nc.all_engine_barrier()
        assert self.sems is not None
        popped = nc._tile_sem_poison_stack.pop()
        assert popped is self._sem_poison
        nc.clear_and_free_semaphores(list(self.sems.allocated().values()))
        nc.all_engine_barrier()


def _split_sync_waits(nc, max_waits=1):
    """This walrus build rejects instructions carrying more than ~2 sync
    waits.  Move excess waits onto same-engine NOPs inserted just before
    the instruction (AND semantics are preserved: the engine blocks on
    each carrier in program order)."""
    for f in nc.m.functions:
        for bb in f.blocks:
            out = []
            for inst in bb.instructions:
                si = inst.sync_info
                waits = list(si.on_wait) if si and si.on_wait else []
                if len(waits) > max_waits:
                    for i in range(0, len(waits) - max_waits, max_waits):
                        nop = mybir.InstNoOp(
                            name=nc.get_next_instruction_name(), ins=[], outs=[]
                        )
                        nop.engine = inst.engine
                        nop.sync_info = mybir.SyncInfo(
                            on_wait=list(waits[i : i + max_waits]), on_update=[]
                        )
                        out.append(nop)
                    si.on_wait[:] = waits[len(waits) - max_waits :]
                out.append(inst)
            bb.instructions[:] = out


# ---------------- device program ----------------
def build_nc(dt_mm=DT_MM):
    nc = bass.Bass(
        "TRN2", target_bir_lowering=False, debug=False, num_devices=N_CORES
    )

    xt = nc.dram_tensor("xt", [D, T], dt_mm, kind="ExternalInput")
    xtq = nc.dram_tensor("xtq", [D, TQ], dt_mm, kind="ExternalInput")
    wqt = nc.dram_tensor("wqt", [D, D], dt_mm, kind="ExternalInput")
    wkt = nc.dram_tensor("wkt", [D, D], dt_mm, kind="ExternalInput")
    wvt = nc.dram_tensor("wvt", [D, D], dt_mm, kind="ExternalInput")
    wot = nc.dram_tensor("wot", [D, D], dt_mm, kind="ExternalInput")
    bq = nc.dram_tensor("bq", [P, NPAIR], F32, kind="ExternalInput")
    bk = nc.dram_tensor("bk", [P, NPAIR], F32, kind="ExternalInput")
    bv = nc.dram_tensor("bv", [P, NPAIR], F32, kind="ExternalInput")
    bob = nc.dram_tensor("bob", [P, D], F32, kind="ExternalInput")
    csaq = nc.dram_tensor("csaq", [P, TQ], F32, kind="ExternalInput")
    csbq = nc.dram_tensor("csbq", [P, TQ], F32, kind="ExternalInput")
    csak = nc.dram_tensor("csak", [P, T], F32, kind="ExternalInput")
    csbk = nc.dram_tensor("csbk", [P, T], F32, kind="ExternalInput")
    out = nc.dram_tensor("out", [TQ, D], F32, kind="ExternalOutput")

    with TileContextSplitDrain(nc) as tc:
        persist_cm = tc.tile_pool(name="persist", bufs=1)
        persist = persist_cm.__enter__()

        def ptile(shape, dt, tag):
            return persist.tile(shape, dt, tag=tag, name=tag)

        with contextlib.ExitStack() as ctx:
            # ---- persistent tiles ----
            csaq_t = ptile([P, TQ], F32, "csaq_t")
            csbq_t = ptile([P, TQ], F32, "csbq_t")
            csak_t = ptile([P, T], F32, "csak_t")
            csbk_t = ptile([P, T], F32, "csbk_t")
            bq_t = ptile([P, NPAIR], F32, "bq_t")
            bk_t = ptile([P, NPAIR], F32, "bk_t")
            bv_t = ptile([P, NPAIR], F32, "bv_t")
            ident = ptile([P, HD], F32, "ident")
            ones64 = ptile([1, HD], F32, "ones64")
            attn = [ptile([P, TQ], dt_mm, f"attn{pr}") for pr in range(NPAIR)]
            nc.sync.dma_start(csaq_t[:], csaq[:])
            nc.sync.dma_start(csbq_t[:], csbq[:])
            nc.sync.dma_start(csak_t[:], csak[:])
            nc.sync.dma_start(csbk_t[:], csbk[:])
            nc.sync.dma_start(bq_t[:], bq[:])
            nc.sync.dma_start(bk_t[:], bk[:])
            nc.sync.dma_start(bv_t[:], bv[:])
            make_identity(nc, ident[0:HD, :])
            make_identity(nc, ident[HD : 2 * HD, :])
            nc.vector.memset(ones64[:], 1.0)

            # ---- pools for the head-pair loop ----
            big = 2 if dt_mm != F32 else 1
            xp = ctx.enter_context(tc.tile_pool(name="xp", bufs=12))
            wp = ctx.enter_context(tc.tile_pool(name="wp", bufs=2))
            rawp = ctx.enter_context(tc.tile_pool(name="rawp", bufs=1))
            ropep = ctx.enter_context(tc.tile_pool(name="ropep", bufs=1))
            vtp = ctx.enter_context(tc.tile_pool(name="vtp", bufs=1))
            qkp = ctx.enter_context(tc.tile_pool(name="qkp", bufs=big))
            vnp = ctx.enter_context(tc.tile_pool(name="vnp", bufs=big))
            exp_p = ctx.enter_context(tc.tile_pool(name="exp_p", bufs=3))
            smallp = ctx.enter_context(tc.tile_pool(name="smallp", bufs=3))
            h1p = ctx.enter_context(tc.tile_pool(name="h1p", bufs=2))
            ps_proj = ctx.enter_context(
                tc.tile_pool(name="ps_proj", bufs=2, space="PSUM")
            )
            ps_vtr = ctx.enter_context(
                tc.tile_pool(name="ps_vtr", bufs=2, space="PSUM")
            )
            ps_sc = ctx.enter_context(
                tc.tile_pool(name="ps_sc", bufs=2, space="PSUM")
            )
            ps_po = ctx.enter_context(
                tc.tile_pool(name="ps_po", bufs=2, space="PSUM")
            )

            def rope(raw, ntok, csa_t, csb_t, out_tile):
                # raw f32 [P, ntok] -> out_tile dt_mm [P, ntok]
                m1 = ropep.tile([P, T], F32, tag="m1", name="m1")
                m2 = ropep.tile([P, T], F32, tag="m2", name="m2")
                t32 = ropep.tile([32, T], F32, tag="t32", name="t32")
                nc.vector.tensor_mul(m1[:, :ntok], raw[:], csa_t[:, :ntok])
                nc.vector.tensor_mul(m2[:, :ntok], raw[:], csb_t[:, :ntok])
                # swap 32-halves within each 64-block of m2 (in place via t32)
                for blk in range(2):
                    b0 = blk * 64
                    nc.vector.tensor_copy(t32[:, :ntok], m2[b0 : b0 + 32, :ntok])
                    nc.vector.tensor_copy(
                        m2[b0 : b0 + 32, :ntok], m2[b0 + 32 : b0 + 64, :ntok]
                    )
                    nc.vector.tensor_copy(
                        m2[b0 + 32 : b0 + 64, :ntok], t32[:, :ntok]
                    )
                nc.vector.tensor_add(out_tile[:], m1[:, :ntok], m2[:, :ntok])

            for pr in range(NPAIR):
                d0 = pr * P
                # -- weight chunk tiles: [feat chunk f][128, 128] --
                wq_c = wp.tile([P, NPAIR, P], dt_mm, tag="wq", name="wq_c")
                wk_c = wp.tile([P, NPAIR, P], dt_mm, tag="wk", name="wk_c")
                wv_c = wp.tile([P, NPAIR, P], dt_mm, tag="wv", name="wv_c")
                for f in range(NPAIR):
                    f0 = f * P
                    nc.sync.dma_start(
                        wq_c[:, f, :], wqt[f0 : f0 + P, d0 : d0 + P]
                    )
                    nc.sync.dma_start(
                        wk_c[:, f, :], wkt[f0 : f0 + P, d0 : d0 + P]
                    )
                    nc.sync.dma_start(
                        wv_c[:, f, :], wvt[f0 : f0 + P, d0 : d0 + P]
                    )

                q_raw = rawp.tile([P, TQ], F32, tag="qraw", name="q_raw")
                k_raw = rawp.tile([P, T], F32, tag="kraw", name="k_raw")
                v_t = vtp.tile([P, T], F32, tag="vt", name="v_t")

                # -- projections, streamed over token blocks --
                for nb in range(T // NQ):
                    t0 = nb * NQ
                    xc = []
                    for f in range(NPAIR):
                        xc_f = xp.tile([P, NQ], dt_mm, tag="xc", name="xc_f")
                        nc.sync.dma_start(
                            xc_f[:], xt[f * P : (f + 1) * P, t0 : t0 + NQ]
                        )
                        xc.append(xc_f)
                    for w_c, b_t, dst, dslice in (
                        (wk_c, bk_t, k_raw, slice(t0, t0 + NQ)),
                        (wv_c, bv_t, v_t, slice(t0, t0 + NQ)),
                    ):
                        ps = ps_proj.tile([P, NQ], F32, tag="ps", name="ps")
                        for f in range(NPAIR):
                            nc.tensor.matmul(
                                ps[:],
                                w_c[:, f, :],
                                xc[f][:],
                                start=(f == 0),
                                stop=(f == NPAIR - 1),
                            )
                        nc.scalar.activation(
                            dst[:, dslice],
                            ps[:],
                            mybir.ActivationFunctionType.Identity,
                            bias=b_t[:, pr : pr + 1],
                        )
                # Q projection over its own query-half token blocks
                for nb in range(TQ // NQ):
                    t0 = nb * NQ
                    xc = []
                    for f in range(NPAIR):
                        xc_f = xp.tile([P, NQ], dt_mm, tag="xc", name="xc_f")
                        nc.sync.dma_start(
                            xc_f[:], xtq[f * P : (f + 1) * P, t0 : t0 + NQ]
                        )
                        xc.append(xc_f)
                    ps = ps_proj.tile([P, NQ], F32, tag="ps", name="ps")
                    for f in range(NPAIR):
                        nc.tensor.matmul(
                            ps[:],
                            wq_c[:, f, :],
                            xc[f][:],
                            start=(f == 0),
                            stop=(f == NPAIR - 1),
                        )
                    nc.scalar.activation(
                        q_raw[:, t0 : t0 + NQ],
                        ps[:],
                        mybir.ActivationFunctionType.Identity,
                        bias=bq_t[:, pr : pr + 1],
                    )

                # -- RoPE --
                qt = qkp.tile([P, TQ], dt_mm, tag="qt", name="qt")
                kt = qkp.tile([P, T], dt_mm, tag="kt", name="kt")
                rope(q_raw, TQ, csaq_t, csbq_t, qt)
                rope(k_raw, T, csak_t, csbk_t, kt)

                # -- V into natural layout + ones column --
                vn = [None, None]
                for hh in range(2):
                    vn_h = vnp.tile(
                        [P, T // P, HD + 1], dt_mm, tag=f"vn{hh}", name="vn_h"
                    )
                    h0 = hh * HD
                    for ch in range(T // P):
                        tp = ps_vtr.tile([P, HD], F32, tag="vtr", name="tp")
                        nc.tensor.transpose(
                            tp[:],
                            v_t[h0 : h0 + HD, ch * P : (ch + 1) * P],
                            ident[h0 : h0 + HD, :],
                        )
                        nc.vector.tensor_copy(vn_h[:, ch, :HD], tp[:])
                    nc.vector.memset(vn_h[:, :, HD : HD + 1], 1.0)
                    vn[hh] = vn_h

                # -- attention per head --
                for hh in range(2):
                    h0 = hh * HD
                    if hh == 0:
                        attn_dst = attn[pr]
                    else:
                        attn_dst = h1p.tile(
                            [HD, TQ], dt_mm, tag="h1", name="attn_dst"
                        )
                    for qb in range(TQ // NQ):
                        qs = slice(qb * NQ, (qb + 1) * NQ)
                        po = ps_po.tile([P, NQ], F32, tag="po", name="po")
                        for ch in range(T // P):
                            ps = ps_sc.tile([P, NQ], F32, tag="sc", name="ps")
                            nc.tensor.matmul(
                                ps[:],
                                kt[h0 : h0 + HD, ch * P : (ch + 1) * P],
                                qt[h0 : h0 + HD, qs],
                                start=True,
                                stop=True,
                            )
                            pexp = exp_p.tile([P, NQ], dt_mm, tag="ex", name="pexp")
                            nc.scalar.activation(
                                pexp[:],
                                ps[:],
                                mybir.ActivationFunctionType.Exp,
                                scale=float(SCALE),
                            )
                            nc.tensor.matmul(
                                po[: HD + 1, :],
                                vn[hh][:, ch, :],
                                pexp[:],
                                start=(ch == 0),
                                stop=(ch == T // P - 1),
                            )
                        rec = smallp.tile([1, NQ], F32, tag="rec", name="rec")
                        nc.vector.reciprocal(rec[:], po[HD : HD + 1, :])
                        pb = ps_sc.tile([P, NQ], F32, tag="sc", name="pb")
                        nc.tensor.matmul(
                            pb[:HD, :], ones64[:], rec[:], start=True, stop=True
                        )
                        recb = smallp.tile([HD, NQ], F32, tag="recb", name="recb")
                        nc.scalar.copy(recb[:], pb[:HD, :])
                        nc.vector.tensor_mul(
                            attn_dst[:HD, qs], po[:HD, :], recb[:]
                        )
                    if hh == 1:
                        # combine odd head into pair region (cross-quadrant
                        # 32-partition copies)
                        nc.vector.tensor_copy(attn[pr][64:96, :], attn_dst[0:32, :])
                        nc.vector.tensor_copy(
                            attn[pr][96:128, :], attn_dst[32:64, :]
                        )

        # ---- output projection (separate pool scope) ----
        with contextlib.ExitStack() as ctx:
            wop = ctx.enter_context(tc.tile_pool(name="wop", bufs=1))
            outp = ctx.enter_context(tc.tile_pool(name="outp", bufs=2))
            ps_o = ctx.enter_context(
                tc.tile_pool(name="ps_o", bufs=2, space="PSUM")
            )
            bob_t = persist.tile([P, D], F32, tag="bob_t", name="bob_t")
            nc.sync.dma_start(bob_t[:], bob[:])
            wo_c = []
            for ch in range(NPAIR):
                wo_ch = wop.tile([P, D], dt_mm, tag=f"wo{ch}", name="wo_ch")
                nc.sync.dma_start(wo_ch[:], wot[ch * P : (ch + 1) * P, :])
                wo_c.append(wo_ch)
            for tb in range(TQ // P):
                ts = slice(tb * P, (tb + 1) * P)
                pout = [
                    ps_o.tile([P, NQ], F32, tag="pout", name="pout")
                    for _ in range(2)
                ]
                for ch in range(NPAIR):
                    for nh in range(2):
                        nc.tensor.matmul(
                            pout[nh][:],
                            attn[ch][:, ts],
                            wo_c[ch][:, nh * NQ : (nh + 1) * NQ],
                            start=(ch == 0),
                            stop=(ch == NPAIR - 1),
                        )
                osb = outp.tile([P, D], F32, tag="osb", name="osb")
                for nh in range(2):
                    nc.vector.tensor_add(
                        osb[:, nh * NQ : (nh + 1) * NQ],
                        pout[nh][:],
                        bob_t[:, nh * NQ : (nh + 1) * NQ],
                    )
                nc.sync.dma_start(out[ts, :], osb[:])

        persist_cm.__exit__(None, None, None)

    _split_sync_waits(nc)
    return nc


# ---------------- host-side input prep ----------------
def _np_dt(dt_mm):
    return ml_dtypes.bfloat16 if dt_mm == mybir.dt.bfloat16 else np.float32


def _cs_tiles(frac_b):
    """csa/csb [128, T] f32 RoPE tiles for one batch (frac_b: [T] f32)."""
    i = np.arange(HALF, dtype=np.float64)
    freq = (ROPE_BASE ** (2.0 * i / HD)).astype(np.float32)  # [32]
    pos = frac_b.astype(np.float32) * np.float32(ROPE_SCALE)
    ang = pos[None, :] / freq[:, None]  # [32, T] f32
    a64 = ang.astype(np.float64)
    cos = np.cos(a64).astype(np.float32)
    sin = np.sin(a64).astype(np.float32)
    csa = np.tile(cos, (4, 1))  # [128, T]
    csb = np.tile(np.concatenate([sin, -sin], axis=0), (2, 1))  # [128, T]
    return np.ascontiguousarray(csa), np.ascontiguousarray(csb)


def make_in_maps(x, frac, Wq, bq, Wk, bk, Wv, bv, Wo, bo, dt_mm=DT_MM):
    npdt = _np_dt(dt_mm)
    wqt = np.ascontiguousarray(Wq.T).astype(npdt)
    wkt = np.ascontiguousarray(Wk.T).astype(npdt)
    wvt = np.ascontiguousarray(Wv.T).astype(npdt)
    wot = np.ascontiguousarray(Wo.T).astype(npdt)
    bq_t = np.ascontiguousarray(bq.reshape(NPAIR, P).T).astype(np.float32)
    bk_t = np.ascontiguousarray(bk.reshape(NPAIR, P).T).astype(np.float32)
    bv_t = np.ascontiguousarray(bv.reshape(NPAIR, P).T).astype(np.float32)
    bob = np.ascontiguousarray(np.tile(bo[None, :], (P, 1))).astype(np.float32)
    in_maps = []
    for c in range(N_CORES):
        b, tqh = c // 2, c % 2
        xt = np.ascontiguousarray(x[b].T).astype(npdt)  # [D, T]
        xtq = np.ascontiguousarray(xt[:, tqh * TQ : (tqh + 1) * TQ])
        csa, csb = _cs_tiles(frac[b])
        in_maps.append(
            {
                "xt": xt,
                "xtq": xtq,
                "wqt": wqt,
                "wkt": wkt,
                "wvt": wvt,
                "wot": wot,
                "bq": bq_t,
                "bk": bk_t,
                "bv": bv_t,
                "bob": bob,
                "csaq": np.ascontiguousarray(csa[:, tqh * TQ : (tqh + 1) * TQ]),
                "csbq": np.ascontiguousarray(csb[:, tqh * TQ : (tqh + 1) * TQ]),
                "csak": csa,
                "csbk": csb,
            }
        )
    return in_maps


_NC_CACHE = {}


def _get_nc(dt_mm=DT_MM):
    key = str(dt_mm)
    if key not in _NC_CACHE:
        _NC_CACHE[key] = build_nc(dt_mm)
    return _NC_CACHE[key]


def kernel(x, frac, Wq, bq, Wk, bk, Wv, bv, Wo, bo):
    install_shims()
    from concourse.bass_utils import run_bass_kernel_spmd

    x = np.asarray(x, dtype=np.float32)
    frac = np.asarray(frac, dtype=np.float32)
    args = [np.asarray(a, dtype=np.float32) for a in (Wq, bq, Wk, bk, Wv, bv, Wo, bo)]
    in_maps = make_in_maps(x, frac, *args, dt_mm=DT_MM)
    nc = _get_nc(DT_MM)
    res = run_bass_kernel_spmd(nc, in_maps, list(range(N_CORES)))
    out = np.empty((B, T, D), dtype=np.float32)
    for c in range(N_CORES):
        b, tqh = c // 2, c % 2
        out[b, tqh * TQ : (tqh + 1) * TQ, :] = res.results[c]["out"]
    return out
